# revision 48
# baseline (speedup 1.0000x reference)
"""Trainium2 Bass kernel for AttnPainterOil-style top-K stroke compositing.

Problem semantics (per pixel, fully independent):
  draw[n] = (n+1) * (alpha[n] > 0.1); top-K=10 of draw over N=256 strokes
  (descending) == the 10 highest-index strokes with alpha > 0.1.  Gather
  alpha/color at those indices and composite back-to-front over a white
  canvas.  Only the top D=20 strokes can enter any pixel's top-10 (host
  verifies the precondition; exact host fallback otherwise).

v2 formulation (dc-telescoping): with T_i the transmittance before stroke i
(T_0 = 1, T_{i+1} = T_i * (1 - aeg_i), aeg = gated effective alpha) the
composite

  canvas = sum_i (T_i - T_{i+1}) c_i + T_20
         = T_0 c_0 + sum_{i=1..19} T_i (c_i - c_{i-1}) + T_20 (1 - c_19)

so with HOST-precomputed color differences dc_i the device never extracts
per-stroke weights ta_i = T_i - T_{i+1}: products use the T planes directly.
The device chain runs sign-alternating (X_i = (-1)^i T_i) so each gated step
is ONE scalar_tensor_tensor (aeg-1)*X; the host bakes the (-1)^i into dc.

Device work:
  * top-k selection: q = 1{alpha>0.1} (ACT sigmoid trick), qualifying-count
    tree + a depth-2 count DAG (pair sums qp, quad sums qq, evens in one
    strided op), gates g = 1{cnt<=9} in two batched ACT ops, aeg = ae*g in
    two batched DVE ops.
  * chain: strokes 0-9 advance in PAIRS X_{2p+2} = X_2p * M_p with
    M_p = am_e*am_o (am = ae-1, host-shipped for the ungated strokes); odd
    planes X_{2p+1} = X_2p * am_e land in ONE strided 5F op.  Strokes 10-19
    advance per stroke via stt (aeg-1)*X (absorbs the -1 for free).
  * products: pr = X (bcast over 3 channels) * dc in big batched ops
    (measured 0.57ns/el with broadcast-middle APs); PE accumulates each
    stroke plane into PSUM via fp16 identity matmuls (c_0 goes to PE
    directly from its HBM tile, no product op).  tc.tile_wait_until pins
    the product ops behind the count path so the Tile scheduler cannot
    hoist them onto DMA-stall positions (the in-order DVE would then block
    the gate-critical count chain behind a waiting product).
  * tail: the last two stroke products (planes 19, 20) skip PE; the output
    is one DVE add of (pr19+pr20) onto the PSUM accumulator, then DMA.

Engine notes kept from v1: ACT only ever runs Sigmoid (one table set,
single ~1.3us load at t~0), PE warmup + mid-kernel keepalive matmuls hold
the HAM clock up, all bulk DMAs ride ONE sync-dispatched HWDGE queue in
strict need-order (a second queue only splits the same ~330GB/s), and
every tight-timing SBUF read is aligned to a single DMA chunk writer.
The walrus NEFF teardown (~6.8us of per-sem clears) is fixed cost.

Sharding: pure data parallel, one batch element per NeuronCore (B=8).
"""

import numpy as np

B, N, W, K = 8, 256, 128, 10
ALPHA_THRESH = 0.1
D = 20          # strokes processed from the top (covers every pixel's
                # top-10 for the target inputs; checked before device path)
P = 128         # partitions (pixel rows)
F = 128         # free dim (pixel cols)
NCORES = 8

# gate = Sigmoid(GATE_SCALE*cnt + GATE_BIAS): cnt<=9 -> 1.0, cnt>=10 -> 0.0
GATE_SCALE = -40.0
GATE_BIAS = 9.5 * 40.0

_nc_cache = {}


def _build_nc(depth):
    import concourse.bass as bass  # noqa: F401
    import concourse.tile as tile
    from concourse import bacc, mybir
    from concourse.vector_clock import ScopedClock

    op = mybir.AluOpType
    f32 = mybir.dt.float32
    f16 = mybir.dt.float16
    actf = mybir.ActivationFunctionType
    assert depth == 20, "emission schedule below is specialized for D=20"

    class _OneShotTileContext(tile.TileContext):
        """TileContext with a slim exit: the drain alone (it waits on the
        global clock, including output-DMA completion) — no all-engine
        barriers and no per-semaphore clears.  Safe because every
        run_bass_kernel_spmd call builds and loads a fresh executable, so
        semaphore state never carries across runs."""

        def _drain_and_barrier(self, tick_clock, wait_clock):
            drain_inst = self.nc.sync.drain()
            wait_clock.add_sem_waits(
                drain_inst.ins, ScopedClock({None: tick_clock.global_clock})
            )
            popped = self.nc._tile_sem_poison_stack.pop()
            assert popped is self._sem_poison

    nc = bacc.Bacc("TRN2", target_bir_lowering=False, debug=False)

    # aeh planes 0-9: am = ae-1 (ungated strokes); planes 10-19: raw ae
    aeh_d = nc.dram_tensor("aeh_in", [P, depth * F], f16, kind="ExternalInput").ap()
    # dc planes: dc_0 = c_0; dcS_i = (-1)^i (c_i - c_{i-1}); dc_20 = 1 - c_19
    dc_d = nc.dram_tensor("dc_in", [P, (depth + 1) * 3 * F], f16,
                          kind="ExternalInput").ap()
    ident_d = nc.dram_tensor("ident_in", [P, P], f16, kind="ExternalInput").ap()
    out_d = nc.dram_tensor("out", [P, 3 * F], f16, kind="ExternalOutput").ap()

    with _OneShotTileContext(nc) as tc:
        with (
            tc.tile_pool(name="const", bufs=1) as constp,
            tc.tile_pool(name="state", bufs=1) as statep,
            tc.tile_pool(name="cntq", bufs=1) as cntqp,
            tc.tile_pool(name="gate", bufs=6) as gatep,
            tc.tile_pool(name="aeg", bufs=6) as aegp,
            tc.tile_pool(name="prod", bufs=5) as prodp,
            tc.tile_pool(name="psum", bufs=1, space="PSUM") as psump,
        ):
            # --- constants / state (all off the DVE critical path) ---
            ident = constp.tile([P, P], f16)
            aeh = statep.tile([P, depth, F], f16)
            dc = statep.tile([P, depth + 1, 3, F], f16)
            X = statep.tile([P, depth + 1, F], f16)   # X_i = (-1)^i T_i
            M04 = statep.tile([P, 5, F], f16)
            qA = statep.tile([P, 10, F], f16)
            qB = statep.tile([P, 10, F], f16)
            qp = statep.tile([P, 4, F], f16)
            qq = statep.tile([P, 2, F], f16)
            s5 = statep.tile([P, 5, F], f16)
            s2 = statep.tile([P, 2, F], f16)
            s1 = statep.tile([P, F], f16)
            warm = statep.tile([P, 1], f16)
            gbias = statep.tile([P, 1], f32)
            qbiasA = statep.tile([P, 1], f32)   # +950 (q from am planes)
            qbiasB = statep.tile([P, 1], f32)   # -50  (q from ae planes)
            zero4 = statep.tile([P, 1], f16)    # PE warmup rhs
            # force the ACT Sigmoid-table load at t~0; every ACT op here is
            # Sigmoid or Copy (copy lives in every table set: no reload)
            nc.scalar.activation(warm[:], warm[:], func=actf.Sigmoid,
                                 bias=gbias[:], scale=GATE_SCALE)

            cacc = psump.tile([P, 3 * F], f32)
            scratch = psump.tile([P, 3 * F], f32)

            # PE warmup off the gpsimd-memset zero tile: HAM clock ramp
            # completes before real compute
            for _ in range(14):
                nc.tensor.matmul(
                    scratch[:, :F], zero4[:].broadcast_to((P, F)),
                    zero4[:].broadcast_to((P, F)),
                    start=True, stop=True, skip_group_check=True,
                )

            def pe_keepalive(n):
                for _ in range(n):
                    nc.tensor.matmul(
                        scratch[:], ident[:],
                        aeh[:, 0:3].rearrange("p s f -> p (s f)"),
                        start=True, stop=True, skip_group_check=True,
                    )

            # --- memsets + all input DMAs, need-ordered ---
            nc.gpsimd.memset(warm[:], 0.0)
            nc.gpsimd.memset(gbias[:], GATE_BIAS)
            nc.gpsimd.memset(qbiasA[:], 950.0)
            nc.gpsimd.memset(qbiasB[:], -50.0)
            nc.gpsimd.memset(zero4[:], 0.0)
            nc.gpsimd.memset(X[:, 0], 1.0)          # X_0 = T_0 = 1
            # two DMA queues in parallel: sync (HWDGE) carries the first half
            # of aeh + aehB + early dc chunks; gpsimd (SWDGE) carries the
            # other aeh half, ident and the late dc chunk.
            def dma_aeh(lo, hi, eng):
                eng.dma_start(
                    aeh[:, lo:hi],
                    aeh_d[:, lo * F: hi * F].rearrange("p (s f) -> p s f", f=F))

            def dma_dc(lo, hi, eng):
                eng.dma_start(
                    dc[:, lo:hi],
                    dc_d[:, lo * 3 * F: hi * 3 * F].rearrange(
                        "p (s c f) -> p s c f", c=3, f=F))

            # parallel HWDGE queues: sync carries the early-need chunks;
            # scalar/vector/tensor queues (their engines are idle at t0)
            # carry aehB and the late dc chunks concurrently.
            # one saturated HWDGE queue in strict need-order (a second queue
            # only steals the same 16 DMA engines' bandwidth); SWDGE carries
            # just the tiny identity matrix.
            dma_aeh(0, 4, nc.sync)
            dma_aeh(4, 10, nc.sync)
            dma_aeh(10, 20, nc.sync)
            nc.gpsimd.dma_start(ident[:], ident_d)
            dma_dc(0, 6, nc.sync)
            dma_dc(6, 11, nc.sync)
            dma_dc(11, 16, nc.sync)
            dma_dc(16, 21, nc.sync)

            # --- ACT stream: q planes, then gates as cnt pairs complete ---
            # q = 1{alpha > 0.1}: planes 0-9 hold am = ae-1 (ae>0 <=> am>-1)
            # so Sigmoid(1000*am + 950) is exactly 0/1; planes 10-18 hold ae.
            # qA in two ops aligned to the two aeh DMA chunks: every
            # tight-timing reader waits on exactly one DMA semaphore
            nc.scalar.activation(
                qA[:, 0:4].rearrange("p s f -> p (s f)"),
                aeh[:, 0:4].rearrange("p s f -> p (s f)"),
                func=actf.Sigmoid, bias=qbiasA[:], scale=1000.0,
            )
            nc.scalar.activation(
                qA[:, 4:10].rearrange("p s f -> p (s f)"),
                aeh[:, 4:10].rearrange("p s f -> p (s f)"),
                func=actf.Sigmoid, bias=qbiasA[:], scale=1000.0,
            )
            nc.scalar.activation(
                qB[:].rearrange("p s f -> p (s f)"),
                aeh[:, 10:20].rearrange("p s f -> p (s f)"),
                func=actf.Sigmoid, bias=qbiasB[:], scale=1000.0,
            )

            # CNTALL plane 2k = cnt_{2k+9} (odd), plane 2k+1 = cnt_{2k+10}
            # (even): plane j is exactly the count gating stroke 10+j.
            cntall = cntqp.tile([P, 10, F], f16)
            G = gatep.tile([P, 10, F], f16)
            AEG = aegp.tile([P, 10, F], f16)

            def gates(lo, hi):
                # gates for strokes 10+lo .. 10+hi-1 in one ACT op
                nc.scalar.activation(
                    G[:, lo:hi].rearrange("p s f -> p (s f)"),
                    cntall[:, lo:hi].rearrange("p s f -> p (s f)"),
                    func=actf.Sigmoid, bias=gbias[:], scale=GATE_SCALE,
                )

            def aeg(lo, hi):
                nc.vector.tensor_tensor(
                    AEG[:, lo:hi].rearrange("p s f -> p (s f)"),
                    aeh[:, 10 + lo: 10 + hi].rearrange("p s f -> p (s f)"),
                    G[:, lo:hi].rearrange("p s f -> p (s f)"), op=op.mult,
                )

            aeh_pairsA = aeh[:, 0:10].rearrange("p (s two) f -> p s two f", two=2)
            am_even = aeh_pairsA[:, :, 0]          # planes 0,2,4,6,8
            am_odd = aeh_pairsA[:, :, 1]           # planes 1,3,5,7,9
            X_evenA = X[:, 0:10].rearrange("p (s two) f -> p s two f", two=2)[:, :, 0]
            X_oddA = X[:, 1:11].rearrange("p (s two) f -> p s two f", two=2)[:, :, 0]

            def chainA(p):
                # X_{2p+2} = X_{2p} * M_p   (all non-negative: pair factors)
                nc.vector.tensor_tensor(
                    X[:, 2 * p + 2], X[:, 2 * p], M04[:, p], op=op.mult)

            def sttB(i):
                # X_{i+1} = (aeg_i - 1) * X_i
                nc.vector.scalar_tensor_tensor(
                    X[:, i + 1], AEG[:, i - 10], 1.0, X[:, i],
                    op0=op.subtract, op1=op.mult)

            def prod(lo, hi, eng=None):
                n = hi - lo
                pr = prodp.tile([P, 5, 3, F], f16, tag="prod", name="prod")
                xb = X[:, lo:hi].unsqueeze(2).broadcast_to((P, n, 3, F))
                (eng or nc.vector).tensor_tensor(
                    pr[:, :n], dc[:, lo:hi], xb, op=op.mult)
                return pr

            def mms(pr, n, ka=0, stop_last=False):
                for j in range(n):
                    nc.tensor.matmul(
                        cacc[:], ident[:],
                        pr[:, j].rearrange("p c f -> p (c f)"),
                        start=False, stop=(stop_last and j == n - 1),
                        skip_group_check=True,
                    )
                if ka:
                    pe_keepalive(ka)

            cnt_pairs = cntall[:].rearrange("p (s two) f -> p s two f", two=2)
            qB_even = qB[:].rearrange("p (s two) f -> p s two f", two=2)[:, :, 0]

            def cnt_odd(k):
                # cnt_{2k+11} = cnt_{2k+9} + (q_{2k+10} + q_{2k+11})
                nc.vector.tensor_tensor(
                    cntall[:, 2 * k + 2], cntall[:, 2 * k], qp[:, k], op=op.add)

            # --- PE: c_0 accumulates straight from the dc tile (no product) ---
            nc.tensor.matmul(
                cacc[:], ident[:], dc[:, 0].rearrange("p c f -> p (c f)"),
                start=True, stop=False, skip_group_check=True,
            )
            pe_keepalive(4)

            # ---------------- DVE emission schedule ----------------
            # Phase A pair chain laced with the count tree; then gated
            # phase B with batched gates/aeg and streamed products.
            nc.vector.tensor_tensor(
                M04[:, 0:2], am_even[:, 0:2], am_odd[:, 0:2], op=op.mult)
            chainA(0)
            nc.vector.tensor_tensor(
                M04[:, 2:5], am_even[:, 2:5], am_odd[:, 2:5], op=op.mult)
            chainA(1)
            chainA(2)
            nc.vector.tensor_tensor(s5[:], qA[:, 0:5], qA[:, 5:10], op=op.add)
            chainA(3)
            nc.vector.tensor_tensor(s2[:], s5[:, 0:2], s5[:, 2:4], op=op.add)
            chainA(4)
            nc.vector.tensor_tensor(s1[:], s2[:, 0], s2[:, 1], op=op.add)
            # odd X planes 1,3,5,7,9 in one strided op (needs X_0..X_8 even)
            nc.vector.tensor_tensor(X_oddA, X_evenA, am_even, op=op.mult)
            nc.vector.tensor_tensor(cntall[:, 0], s1[:], s5[:, 4], op=op.add)
            # depth-2 count DAG over the gated strokes: pair sums qp, quad
            # sums qq, then every odd count is <=2 adds from cnt_9 and
            # every even count is its odd neighbour + one q.
            nc.vector.tensor_tensor(
                qp[:],
                qB[:, 0:8].rearrange("p (s two) f -> p s two f", two=2)[:, :, 0],
                qB[:, 0:8].rearrange("p (s two) f -> p s two f", two=2)[:, :, 1],
                op=op.add,
            )
            nc.vector.tensor_tensor(
                qq[:],
                qp[:].rearrange("p (s two) f -> p s two f", two=2)[:, :, 0],
                qp[:].rearrange("p (s two) f -> p s two f", two=2)[:, :, 1],
                op=op.add)
            nc.vector.tensor_tensor(cntall[:, 2], cntall[:, 0], qp[:, 0], op=op.add)   # cnt_11
            nc.vector.tensor_tensor(cntall[:, 1], cntall[:, 0], qB[:, 0], op=op.add)   # cnt_10
            nc.vector.tensor_tensor(cntall[:, 4], cntall[:, 0], qq[:, 0], op=op.add)   # cnt_13
            nc.vector.tensor_tensor(cntall[:, 3], cntall[:, 2], qB[:, 2], op=op.add)   # cnt_12
            gates(0, 4)          # strokes 10-13: first gate fires ~1us sooner
            nc.vector.tensor_tensor(cntall[:, 8], cntall[:, 4], qq[:, 1], op=op.add)   # cnt_17
            nc.vector.tensor_tensor(cntall[:, 6], cntall[:, 4], qp[:, 2], op=op.add)   # cnt_15
            # cnt_14/16/18 = cnt_13/15/17 + q_14/16/18 in one strided op
            nc.vector.tensor_tensor(
                cnt_pairs[:, 2:5, 1], cnt_pairs[:, 2:5, 0], qB_even[:, 2:5], op=op.add)
            gates(4, 10)         # strokes 14-19
            aeg(0, 4)
            sttB(10)
            with tc.tile_wait_until(0.007):
                prA1 = prod(1, 6)
            mms(prA1, 5, ka=2)
            sttB(11)
            aeg(4, 10)
            sttB(12)
            with tc.tile_wait_until(0.008):
                prA2 = prod(6, 11)
            mms(prA2, 5, ka=2)
            sttB(13)
            sttB(14)
            with tc.tile_wait_until(0.010):
                prB1 = prod(11, 15)
            mms(prB1, 4, ka=2)
            sttB(15)
            sttB(16)
            sttB(17)
            sttB(18)
            with tc.tile_wait_until(0.012):
                prB2 = prod(15, 19)
            mms(prB2, 4, stop_last=True)
            sttB(19)
            with tc.tile_wait_until(0.014):
                pr1920 = prod(19, 21)
            # tail on DVE: out = (pr_19 + pr_20) + cacc; the last two stroke
            # products never visit PE, so no stop-mm -> ACT hop at the end
            prsum = constp.tile([P, 3, F], f16, tag="prsum")
            nc.vector.tensor_tensor(prsum[:], pr1920[:, 0], pr1920[:, 1], op=op.add)
            out_t = constp.tile([P, 3, F], f16, tag="out")
            nc.vector.tensor_tensor(
                out_t[:], cacc[:].rearrange("p (c f) -> p c f", c=3), prsum[:],
                op=op.add,
            )
            nc.sync.dma_start(out_d, out_t[:].rearrange("p c f -> p (c f)"))

    nc.compile()
    return nc


def _prep_inputs(color_stroke, alpha, depth):
    """Host prep: slice the top `depth` strokes (reversed: stroke 0 = highest
    index), resolve the alpha threshold in f32, and lay out per core in fp16:

      aeh [P, depth*F]:  planes 0-9  = am  = ae - 1   (ungated strokes)
                         planes 10-19 = ae             (gate applied on device)
      dc  [P, 21*3*F]:   dc_0 = c_0; dcS_i = (-1)^i (c_i - c_{i-1});
                         dc_20 = 1 - c_19   (white background fold)
    """
    a_r = alpha[:, N - depth:, 0][:, ::-1]               # (B, depth, P, F) f32
    ae0 = (a_r * (a_r > ALPHA_THRESH)).astype(np.float32)
    aeh = np.empty((B, depth, P, F), np.float16)
    aeh[:, :10] = (ae0[:, :10] - 1.0).astype(np.float16)
    aeh[:, 10:] = ae0[:, 10:].astype(np.float16)

    c_r = color_stroke[:, N - depth:][:, ::-1].astype(np.float32)  # (B,depth,3,P,F)
    dc = np.empty((B, depth + 1, 3, P, F), np.float32)
    dc[:, 0] = c_r[:, 0]
    dc[:, 1:depth] = c_r[:, 1:] - c_r[:, :-1]
    dc[:, depth] = 1.0 - c_r[:, depth - 1]
    dc[:, 1:depth:2] = -dc[:, 1:depth:2]                 # odd strokes negated
    dc16 = dc.astype(np.float16)

    ident = np.eye(P, dtype=np.float16)
    in_maps = []
    for b in range(B):
        a_core = np.ascontiguousarray(
            aeh[b].transpose(1, 0, 2)).reshape(P, depth * F)
        d_core = np.ascontiguousarray(
            dc16[b].transpose(2, 0, 1, 3)).reshape(P, (depth + 1) * 3 * F)
        in_maps.append({"aeh_in": a_core, "dc_in": d_core, "ident_in": ident})
    return in_maps


def _reference_numpy(color_stroke, alpha):
    """Exact replication of the oracle (incl. top-k tie-breaking) on host.
    Only used when the depth-cutoff precondition fails (pathological inputs)."""
    stroke_ids = np.arange(1, N + 1, dtype=np.int32).reshape(1, N, 1, 1)
    draw = stroke_ids * (alpha[:, :, 0] > ALPHA_THRESH).astype(np.int32)
    draw_t = np.moveaxis(draw, 1, -1)
    idx = np.argsort(-draw_t, axis=-1, kind="stable")[..., :K]
    idx = np.moveaxis(idx, -1, 1)[:, :, None]
    alpha_k = np.take_along_axis(alpha, idx, axis=1)
    color_k = np.take_along_axis(color_stroke, idx, axis=1)
    canvas = np.ones((B, 3, W, W), dtype=color_stroke.dtype)
    for i in range(K - 1, -1, -1):
        a = alpha_k[:, i]
        canvas = canvas * (1.0 - a) + a * color_k[:, i]
    return canvas


def kernel(color_stroke, alpha):
    color_stroke = np.asarray(color_stroke, dtype=np.float32)
    alpha = np.asarray(alpha, dtype=np.float32)
    assert color_stroke.shape == (B, N, 3, W, W), color_stroke.shape
    assert alpha.shape == (B, N, 1, W, W), alpha.shape

    # Precondition for the depth cutoff: every pixel finds its 10 passing
    # strokes within the top D.
    top_pass = (alpha[:, N - D:, 0] > ALPHA_THRESH).sum(axis=1)
    if top_pass.min() < K:
        return _reference_numpy(color_stroke, alpha)

    from concourse.bass_utils import run_bass_kernel_spmd

    if D not in _nc_cache:
        _nc_cache[D] = _build_nc(D)
    nc = _nc_cache[D]

    in_maps = _prep_inputs(color_stroke, alpha, D)
    res = run_bass_kernel_spmd(nc, in_maps, core_ids=list(range(NCORES)))

    out = np.empty((B, 3, W, W), dtype=np.float32)
    for b in range(B):
        out[b] = (
            res.results[b]["out"].astype(np.float32).reshape(P, 3, F).transpose(1, 0, 2)
        )
    return out
